# revision 20
# baseline (speedup 1.0000x reference)
# Multi-head attention kernel for Trainium2 (8 NeuronCores, SPMD).
#
# Problem (hardcoded): X[4, 2048, 1024], W_k/W_q/W_v/W_u[1024, 1024], b_u[1024]
#   K = (X @ W_k.T) * s ; Q = (X @ W_q.T) * s ; V = (X @ W_v.T) * s   (s = 1024**-0.25)
#   S = Q @ K.T per head (16 heads, head_dim 64); P = softmax(S); Y = P @ V
#   out = Y @ W_u.T + b_u
#
# Sharding: core c = (batch c//2, head-half c%2). Each core computes K/Q/V for
# its 8 heads over the full sequence of its batch and the matching 512-wide
# slice of the output projection, producing a PARTIAL output [2048, 1024]
# (feature-contraction split). Host unshard = add the two partials per batch;
# b_u is added on the hh==0 core only.
#
# Design notes (from trace analysis of earlier versions):
#   - exp on ScalarE is the hard floor (~295us/core: 33.5M score elements at
#     1 elem/cycle/lane @1.2GHz, invariant under sharding). Everything else
#     is scheduled to hide under it.
#   - Engine queues are in-order; the emission order IS the per-engine
#     execution order. All PE work is therefore interleaved at the
#     granularity of one score tile (one exp call).
#   - The NEFF is compiled with LDW-opt off, so a matmul whose stationary
#     operand differs from the previous one serializes on the weight load.
#     The hardware DOES pull LDWEIGHTS ahead for disjoint row groups, so
#     every full-row matmul here is split into two 64-row halves on
#     alternating row groups (concurrent streams, weights loads hidden);
#     the halves' partial sums are combined by the DVE add that replaces
#     the PSUM-eviction copy.
#   - Uniform lag-one-quarter schedule: the attention q axis is processed in
#     512-wide quarters; quarter t's AV matmuls are interleaved into quarter
#     t+1's slot loop (a deep pts ring carries the exp outputs across). The
#     two quarters of slack this opens at the start absorb the V projection;
#     quarter boundaries absorb K/Q prefetch chains and the output
#     projection of the last pair.
#   - Softmax denominators ride as row 64 of the AV output (ones column in
#     V); their reciprocal is computed partition-packed ([128, 8] via a DRAM
#     bounce) instead of on a 1-partition row (pathologically slow on DVE).
#
# PSUM budget (8 banks): score psum "ps" [128,1024] x2 (4 banks) +
# "av" [128,512] x4 (4 banks; quarter AV accumulator halves, and between
# quarters the projection/V/output chain halves).

import numpy as np
import ml_dtypes

import concourse.bacc as bacc
import concourse.mybir as mybir
import concourse.tile as tile
from concourse.bass_utils import run_bass_kernel_spmd

FP32 = mybir.dt.float32
BF16 = mybir.dt.bfloat16
AF = mybir.ActivationFunctionType
BF16NP = ml_dtypes.bfloat16

P = 128
E = 1024            # embedding dim
T = 2048            # sequence length
HC = 8              # heads per core
S = 64              # head dim
ET = E // P         # 8 contraction tiles over e
KT = T // P         # 16 key tiles
NPAIR = HC // 2     # 4 head pairs per core
QW = 512            # query-quarter width
NQ = T // QW        # 4 query quarters per pair
FC = NPAIR          # feature k-tiles for the output projection
SCALE = float(E ** -0.25)

N_CORES = 8


def build_nc():
    nc = bacc.Bacc("TRN2", target_bir_lowering=False, debug=False,
                   enable_asserts=False)

    xt = nc.dram_tensor("xt", [E, T], BF16, kind="ExternalInput").ap()
    wk = nc.dram_tensor("wk", [E, P * NPAIR], BF16, kind="ExternalInput").ap()
    wq = nc.dram_tensor("wq", [E, P * NPAIR], BF16, kind="ExternalInput").ap()
    wv = nc.dram_tensor("wv", [E, P * NPAIR], BF16, kind="ExternalInput").ap()
    wu = nc.dram_tensor("wu", [P * NPAIR, E], BF16, kind="ExternalInput").ap()
    bu = nc.dram_tensor("bu", [1, E], FP32, kind="ExternalInput").ap()
    out = nc.dram_tensor("out", [T, E], FP32, kind="ExternalOutput").ap()

    with tile.TileContext(nc) as tc:
        _build_kernel(tc, nc, xt, wk, wq, wv, wu, bu, out)
    nc.compile()
    return nc


def _build_kernel(tc, nc, xt, wk, wq, wv, wu, bu, out):
    with (
        tc.tile_pool(name="main", bufs=1) as mp,
        tc.tile_pool(name="psum", bufs=1, space="PSUM") as pspool,
        tc.tile_pool(name="dram", bufs=1, space="DRAM") as drampool,
    ):
        vv = mp.tile([P, KT, HC, S + 1], BF16, tag="vv", name="vv")
        yt = mp.tile([P, FC, T], BF16, tag="yt", name="yt")
        bub = mp.tile([P, E], FP32, tag="bub", name="bub")
        xt_sb = mp.tile([P, ET, T], BF16, tag="xtb", name="xtb")

        def avt(name):
            return pspool.tile([P, QW], FP32, tag="av", bufs=4, name=name)

        def psum_combine(dst_ap, h0_ap, h1_ap, nm, rows=P):
            # DVE reads at most one PSUM operand per instruction: stage one
            # half in SBUF, then add
            cmb = mp.tile([P, QW], FP32, tag="cmb", bufs=3, name=f"cmb{nm}")
            nc.vector.tensor_copy(out=cmb[0:rows, :], in_=h1_ap)
            nc.vector.tensor_add(out=dst_ap, in0=h0_ap, in1=cmb[0:rows, :])

        def load_w(dram_ap, j, tag, name):
            wt = mp.tile([P, ET, P], BF16, tag=tag, bufs=2, name=name)
            nc.sync.dma_start(
                wt[:], dram_ap[:, j * P:(j + 1) * P].rearrange(
                    "(ko p) m -> p ko m", p=P))
            return wt

        def proj_chunk(wb, dst, t0, uname):
            """dst[:, t0:t0+512] = wb.T @ X^T chunk, via two 64-row
            half-chains on alternating row groups (packed into one
            score-ring tile) + a DVE combine."""
            ph = pspool.tile([P, 2 * QW], FP32, tag="ps", bufs=2,
                             name=f"pp{uname}_{t0}")
            for k in range(ET):
                for h in range(2):
                    nc.tensor.matmul(
                        ph[:, h * QW:(h + 1) * QW],
                        lhsT=wb[h * S:(h + 1) * S, k, :],
                        rhs=xt_sb[h * S:(h + 1) * S, k, t0:t0 + QW],
                        start=(k == 0), stop=(k == ET - 1))
            psum_combine(dst[:, t0:t0 + QW], ph[:, 0:QW], ph[:, QW:2 * QW],
                         f"p{uname}{t0}")

        def proj_chunk_av(wb, dst, t0, uname):
            """Same, on the av-ring (startup only, before the AV
            accumulators exist)."""
            ph = [avt(f"pa{uname}_{t0}_{h}") for h in range(2)]
            for k in range(ET):
                for h in range(2):
                    nc.tensor.matmul(
                        ph[h][:],
                        lhsT=wb[h * S:(h + 1) * S, k, :],
                        rhs=xt_sb[h * S:(h + 1) * S, k, t0:t0 + QW],
                        start=(k == 0), stop=(k == ET - 1))
            psum_combine(dst[:, t0:t0 + QW], ph[0][:], ph[1][:],
                         f"p{uname}{t0}")

        def v_finish(mt, in0, in1, both_psum=True):
            vs = mp.tile([P, QW], FP32, tag="vs", bufs=2, name=f"vs{mt}")
            if both_psum:
                psum_combine(vs[:], in0, in1, f"v{mt}")
            else:
                nc.vector.tensor_add(out=vs[:], in0=in0, in1=in1)
            nc.vector.tensor_copy(
                out=vv[:, mt, :, 0:S],
                in_=vs[:].rearrange("p (h s) -> p h s", s=S))
            nc.vector.memset(vv[:, mt, :, S:S + 1], 1.0)

        def v_chunk(mt):
            """vv tile mt (one 128-token block, 512 features + ones col),
            halves on the av-ring (quarter 0 only — ring is free there)."""
            ph = [avt(f"pv{mt}_{h}") for h in range(2)]
            for k in range(ET):
                for h in range(2):
                    nc.tensor.matmul(
                        ph[h][:],
                        lhsT=xt_sb[h * S:(h + 1) * S, k,
                                   mt * P:(mt + 1) * P],
                        rhs=wvb[h * S:(h + 1) * S, k, :],
                        start=(k == 0), stop=(k == ET - 1))
            v_finish(mt, ph[0][:], ph[1][:])

        def v_chunk_ps(mt):
            """Same, but halves packed into one score-ring tile (quarter 1:
            the av-ring is held by the lagged AV accumulators there)."""
            ph = pspool.tile([P, 2 * QW], FP32, tag="ps", bufs=2,
                             name=f"pvp{mt}")
            for k in range(ET):
                for h in range(2):
                    nc.tensor.matmul(
                        ph[:, h * QW:(h + 1) * QW],
                        lhsT=xt_sb[h * S:(h + 1) * S, k,
                                   mt * P:(mt + 1) * P],
                        rhs=wvb[h * S:(h + 1) * S, k, :],
                        start=(k == 0), stop=(k == ET - 1))
            v_finish(mt, ph[:, 0:QW], ph[:, QW:2 * QW])

        def out_unit(m, n0, ot):
            """ot[:, n0:n0+512] = out-projection chunk for token tile m;
            contraction halves packed into one score-ring tile."""
            ph = pspool.tile([P, 2 * QW], FP32, tag="ps", bufs=2,
                             name=f"po{m}_{n0}")
            for k in range(FC):
                for h in range(2):
                    nc.tensor.matmul(
                        ph[:, h * QW:(h + 1) * QW],
                        lhsT=yt[h * S:(h + 1) * S, k, m * P:(m + 1) * P],
                        rhs=wub[h * S:(h + 1) * S, k, n0:n0 + QW],
                        start=(k == 0), stop=(k == FC - 1))
            tsum = mp.tile([P, QW], FP32, tag="osum", bufs=2,
                           name=f"os{m}_{n0}")
            psum_combine(tsum[:], ph[:, 0:QW], ph[:, QW:2 * QW],
                         f"o{m}_{n0}")
            nc.vector.tensor_add(out=ot[:, n0:n0 + QW], in0=tsum[:],
                                 in1=bub[:, n0:n0 + QW])

        def emit_out_tile(m):
            ot = mp.tile([P, E], FP32, tag="ot", bufs=2, name=f"ot{m}")
            for n0 in (0, QW):
                out_unit(m, n0, ot)
            nc.sync.dma_start(out[m * P:(m + 1) * P, :], ot[:])

        def emit_normalize(j, qq, avh):
            # combine AV halves (also evicts the banks), batch the two
            # denominator rows into a [128, 8] partition-packed reciprocal
            # via a DRAM bounce, broadcast back, normalize into yt
            q0 = qq * QW
            yraws = []
            for par in range(2):
                yraw = mp.tile([S + 1, QW], FP32, tag=f"yraw{par}",
                               bufs=2, name=f"yraw{j}_{qq}_{par}")
                psum_combine(yraw[:], avh[par][0][0:S + 1, :],
                             avh[par][1][0:S + 1, :], f"n{j}_{qq}_{par}",
                             rows=S + 1)
                yraws.append(yraw)
            db = drampool.tile([1, 2 * QW], FP32, tag="db", bufs=2,
                               name=f"db{j}_{qq}")
            for par in range(2):
                nc.sync.dma_start(db[:, par * QW:(par + 1) * QW],
                                  yraws[par][S:S + 1, :])
            rin = mp.tile([P, 8], FP32, tag="rin", bufs=2,
                          name=f"rin{j}_{qq}")
            nc.sync.dma_start(
                rin[:], db[0:1, :].rearrange("a (p f) -> (a p) f", p=P))
            rcp = mp.tile([P, 8], FP32, tag="rcp", bufs=2,
                          name=f"rcp{j}_{qq}")
            nc.vector.reciprocal_approx_fast(out=rcp[:], in_=rin[:])
            db2 = drampool.tile([1, 2 * QW], FP32, tag="db2", bufs=2,
                                name=f"db2{j}_{qq}")
            nc.sync.dma_start(
                db2[0:1, :].rearrange("a (p f) -> (a p) f", p=P), rcp[:])
            for par in range(2):
                rbc = mp.tile([S, QW], FP32, tag="rbc", bufs=2,
                              name=f"rbc{j}_{qq}_{par}")
                nc.sync.dma_start(
                    rbc[:],
                    db2[0:1, par * QW:(par + 1) * QW].to_broadcast([S, QW]))
                if par == 0:
                    nc.vector.tensor_mul(out=yt[0:S, j, q0:q0 + QW],
                                         in0=yraws[par][0:S, :], in1=rbc[:])
                else:
                    tmp = mp.tile([S, QW], BF16, tag="tmp", bufs=2,
                                  name=f"tmp{j}_{qq}")
                    nc.vector.tensor_mul(out=tmp[:],
                                         in0=yraws[par][0:S, :], in1=rbc[:])
                    nc.sync.dma_start(yt[S:P, j, q0:q0 + QW], tmp[:])

        # --- input DMAs
        wkj = {0: load_w(wk, 0, "wkj", "wk0")}
        wqj = {0: load_w(wq, 0, "wqj", "wq0")}
        for k in range(ET):
            nc.sync.dma_start(xt_sb[:, k, :], xt[k * P:(k + 1) * P, :])
        wvb = mp.tile([P, ET, P * NPAIR], BF16, tag="wvb", name="wvb")
        nc.sync.dma_start(wvb[:], wv.rearrange("(ko p) m -> p ko m", p=P))
        nc.sync.dma_start(bub[:], bu.to_broadcast([P, E]))
        wub = mp.tile([P, FC, E], BF16, tag="wub", name="wub")
        nc.sync.dma_start(wub[:], wu.rearrange("(ko p) m -> p ko m", p=P))

        kq = {0: (mp.tile([P, T], BF16, tag="ktj", bufs=2, name="kt0"),
                  mp.tile([P, T], BF16, tag="qtj", bufs=2, name="qt0"))}

        # pair-0 K/Q startup chains on the av-ring (first score needs only
        # kt c0 + qt c0, so emit those first)
        proj_chunk_av(wkj[0], kq[0][0], 0, "k0")
        proj_chunk_av(wqj[0], kq[0][1], 0, "q0")
        for t0 in range(QW, T, QW):
            proj_chunk_av(wkj[0], kq[0][0], t0, "k0")

        # background work queues -------------------------------------------
        # mid_fill[t] = thunks emitted inside quarter t at slots 6 and 11
        QTOT = NPAIR * NQ  # 16 quarters
        mid_fill = [[] for _ in range(QTOT)]

        def defer(t, fn):
            mid_fill[t].append(fn)

        # pair-0 remaining Q chunks: quarter t (=qq) needs qt chunk qq
        for c in (1, 2, 3):
            defer(c - 1, lambda c=c: proj_chunk(wqj[0], kq[0][1], c * QW,
                                                "q0"))
        # pairs 1..3: weights DMA + kt c0-c3 + qt c0 before quarter 4*jn;
        # qt c1-c3 before their quarters
        for jn in (1, 2, 3):
            base = 4 * (jn - 1)

            def mk_w(jn=jn):
                wkj[jn] = load_w(wk, jn, "wkj", f"wk{jn}")
                wqj[jn] = load_w(wq, jn, "wqj", f"wq{jn}")
                kq[jn] = (mp.tile([P, T], BF16, tag="ktj", bufs=2,
                                  name=f"kt{jn}"),
                          mp.tile([P, T], BF16, tag="qtj", bufs=2,
                                  name=f"qt{jn}"))
            defer(base, mk_w)
            defer(base + 1, lambda jn=jn: proj_chunk(wkj[jn], kq[jn][0], 0,
                                                     f"k{jn}"))
            defer(base + 1, lambda jn=jn: proj_chunk(wkj[jn], kq[jn][0], QW,
                                                     f"k{jn}"))
            defer(base + 2, lambda jn=jn: proj_chunk(wkj[jn], kq[jn][0],
                                                     2 * QW, f"k{jn}"))
            defer(base + 2, lambda jn=jn: proj_chunk(wkj[jn], kq[jn][0],
                                                     3 * QW, f"k{jn}"))
            defer(base + 3, lambda jn=jn: proj_chunk(wqj[jn], kq[jn][1], 0,
                                                     f"q{jn}"))
            for c in (1, 2, 3):
                defer(4 * jn + c - 1,
                      lambda jn=jn, c=c: proj_chunk(wqj[jn], kq[jn][1],
                                                    c * QW, f"q{jn}"))

        # --- quarter loop: uniform lag-one-quarter schedule ---------------
        prev = None  # (j, qq, pts list) whose AVs run in this quarter
        for t in range(QTOT):
            j, qq = t // NQ, t % NQ
            q0 = qq * QW
            ktj, qtj = kq[j]
            pts = []
            avh = None
            if prev is not None:
                avh = [[avt(f"av{t}_{par}_{h}") for h in range(2)]
                       for par in range(2)]
            # out-projection units for the last pair's quarters, two
            # normalizes behind (their yt slice is complete by then)
            outq = []
            if j == NPAIR - 1 and qq >= 2:
                for m in range((qq - 2) * NQ, (qq - 1) * NQ):
                    ot = mp.tile([P, E], FP32, tag="ot", bufs=2,
                                 name=f"ot{m}")
                    outq.append((m, 0, ot, False))
                    outq.append((m, QW, ot, True))
            fills = list(mid_fill[t])
            nf = len(fills)
            for i in range(KT):
                ps = pspool.tile([P, 2 * QW], FP32, tag="ps", bufs=2,
                                 name=f"s{t}_{i}")
                for par in range(2):
                    lo = par * S
                    nc.tensor.matmul(
                        ps[:, par * QW:(par + 1) * QW],
                        lhsT=ktj[lo:lo + S, i * P:(i + 1) * P],
                        rhs=qtj[lo:lo + S, q0:q0 + QW],
                        start=True, stop=True)
                pt = mp.tile([P, 2 * QW], BF16, tag="pt", bufs=20,
                             name=f"p{t}_{i}")
                nc.scalar.activation(pt[:], ps[:], AF.Exp)
                pts.append(pt)
                # V projection fills the slack of the first two quarters
                # (emitted before the AVs that will consume those vv tiles)
                if t == 0 and i % 2 == 0:
                    v_chunk(i // 2)
                elif t == 1 and i % 2 == 0:
                    v_chunk_ps(8 + i // 2)
                if prev is not None:
                    # lagged AV for the previous quarter, same key tile i
                    pj, pqq, ppts = prev
                    for par in range(2):
                        h2 = 2 * pj + par
                        for hf in range(2):
                            nc.tensor.matmul(
                                avh[par][hf][0:S + 1, :],
                                lhsT=vv[hf * S:(hf + 1) * S, i, h2, :],
                                rhs=ppts[i][hf * S:(hf + 1) * S,
                                            par * QW:(par + 1) * QW],
                                start=(i == 0), stop=(i == KT - 1))
                # background projection chunks, mid-quarter
                if i == 6:
                    for fn in fills[:nf // 2]:
                        fn()
                elif i == 11:
                    for fn in fills[nf // 2:]:
                        fn()
                # one output-projection unit per odd slot
                if outq and i % 2 == 1:
                    m, n0, ot, last = outq.pop(0)
                    out_unit(m, n0, ot)
                    if last:
                        nc.sync.dma_start(out[m * P:(m + 1) * P, :], ot[:])
            # leftover output units (none expected in steady quarters)
            for m, n0, ot, last in outq:
                out_unit(m, n0, ot)
                if last:
                    nc.sync.dma_start(out[m * P:(m + 1) * P, :], ot[:])
            # boundary: previous quarter's normalize (DVE/DMA only)
            if prev is not None:
                emit_normalize(prev[0], prev[1], avh)
            prev = (j, qq, pts)

        # --- tail: last quarter's AVs, normalize, final output tiles
        j, qq, ppts = prev
        avh = [[avt(f"avT_{par}_{h}") for h in range(2)]
               for par in range(2)]
        for i in range(KT):
            for par in range(2):
                h2 = 2 * j + par
                for hf in range(2):
                    nc.tensor.matmul(
                        avh[par][hf][0:S + 1, :],
                        lhsT=vv[hf * S:(hf + 1) * S, i, h2, :],
                        rhs=ppts[i][hf * S:(hf + 1) * S,
                                    par * QW:(par + 1) * QW],
                        start=(i == 0), stop=(i == KT - 1))
        emit_normalize(j, qq, avh)
        for m in range((NQ - 2) * NQ, NQ * NQ):
            emit_out_tile(m)


_NC_CACHE = {}


def _get_nc():
    if "nc" not in _NC_CACHE:
        _NC_CACHE["nc"] = build_nc()
    return _NC_CACHE["nc"]


def make_in_maps(X, W_k, W_q, W_v, W_u, b_u):
    X = np.asarray(X, np.float32)
    b = X.shape[0]
    HW = P * NPAIR  # 512 features per head-half
    wk_t = (np.asarray(W_k, np.float32).T * SCALE).astype(BF16NP)
    wq_t = (np.asarray(W_q, np.float32).T * SCALE).astype(BF16NP)
    wv_t = (np.asarray(W_v, np.float32).T * SCALE).astype(BF16NP)
    wu_t = np.asarray(W_u, np.float32).T.astype(BF16NP)
    bu2 = np.ascontiguousarray(np.asarray(b_u, np.float32).reshape(1, E))
    bu_zero = np.zeros((1, E), np.float32)
    wk_s = [np.ascontiguousarray(wk_t[:, hh * HW:(hh + 1) * HW])
            for hh in range(2)]
    wq_s = [np.ascontiguousarray(wq_t[:, hh * HW:(hh + 1) * HW])
            for hh in range(2)]
    wv_s = [np.ascontiguousarray(wv_t[:, hh * HW:(hh + 1) * HW])
            for hh in range(2)]
    wu_s = [np.ascontiguousarray(wu_t[hh * HW:(hh + 1) * HW, :])
            for hh in range(2)]
    xts = [np.ascontiguousarray(X[bi].T).astype(BF16NP) for bi in range(b)]
    in_maps = []
    for c in range(N_CORES):
        bi, hh = c // 2, c % 2
        in_maps.append({
            "xt": xts[bi],
            "wk": wk_s[hh], "wq": wq_s[hh], "wv": wv_s[hh],
            "wu": wu_s[hh],
            "bu": bu2 if hh == 0 else bu_zero,
        })
    return in_maps


def run(inputs, trace=False, **kwargs):
    """Run on hardware; returns (full output, BassKernelResults)."""
    X = np.asarray(inputs["X"], np.float32)
    b, t, e = X.shape
    nc = _get_nc()
    in_maps = make_in_maps(X, inputs["W_k"], inputs["W_q"], inputs["W_v"],
                           inputs["W_u"], inputs["b_u"])
    res = run_bass_kernel_spmd(nc, in_maps, core_ids=list(range(N_CORES)),
                               trace=trace, **kwargs)
    full = np.empty((b, t, e), np.float32)
    for bi in range(b):
        full[bi] = res.results[2 * bi]["out"] + res.results[2 * bi + 1]["out"]
    return full, res


def kernel(**inputs):
    full, _ = run(inputs)
    return full


# revision 23
# speedup vs baseline: 1.0320x; 1.0320x over previous
# Multi-head attention kernel for Trainium2 (8 NeuronCores, SPMD).
#
# Problem (hardcoded): X[4, 2048, 1024], W_k/W_q/W_v/W_u[1024, 1024], b_u[1024]
#   K = (X @ W_k.T) * s ; Q = (X @ W_q.T) * s ; V = (X @ W_v.T) * s   (s = 1024**-0.25)
#   S = Q @ K.T per head (16 heads, head_dim 64); P = softmax(S); Y = P @ V
#   out = Y @ W_u.T + b_u
#
# Sharding: core c = (batch c//2, head-half c%2). Each core computes K/Q/V for
# its 8 heads over the full sequence of its batch and the matching 512-wide
# slice of the output projection, producing a PARTIAL output [2048, 1024]
# (feature-contraction split). Host unshard = add the two partials per batch;
# b_u is added on the hh==0 core only.
#
# Design notes (from trace analysis of earlier versions):
#   - exp on ScalarE is the hard floor (~295us/core: 33.5M score elements at
#     1 elem/cycle/lane @1.2GHz, invariant under sharding). Everything else
#     is scheduled to hide under it.
#   - Engine queues are in-order; the emission order IS the per-engine
#     execution order. All PE work is therefore interleaved at the
#     granularity of one score tile (one exp call).
#   - The NEFF is compiled with LDW-opt off, so a matmul whose stationary
#     operand differs from the previous one serializes on the weight load.
#     The hardware DOES pull LDWEIGHTS ahead for disjoint row groups, so
#     every full-row matmul here is split into two 64-row halves on
#     alternating row groups (concurrent streams, weights loads hidden);
#     the halves' partial sums are combined by the DVE add that replaces
#     the PSUM-eviction copy.
#   - Uniform lag-one-quarter schedule: the attention q axis is processed in
#     512-wide quarters; quarter t's AV matmuls are interleaved into quarter
#     t+1's slot loop (a deep pts ring carries the exp outputs across). The
#     two quarters of slack this opens at the start absorb the V projection;
#     quarter boundaries absorb K/Q prefetch chains and the output
#     projection of the last pair.
#   - Softmax denominators ride as row 64 of the AV output (ones column in
#     V); their reciprocal is computed partition-packed ([128, 8] via a DRAM
#     bounce) instead of on a 1-partition row (pathologically slow on DVE).
#
# PSUM budget (8 banks): score psum "ps" [128,1024] x2 (4 banks) +
# "av" [128,512] x4 (4 banks; quarter AV accumulator halves, and between
# quarters the projection/V/output chain halves).

import numpy as np
import ml_dtypes

import concourse.bacc as bacc
import concourse.mybir as mybir
import concourse.tile as tile
from concourse.bass_utils import run_bass_kernel_spmd

FP32 = mybir.dt.float32
BF16 = mybir.dt.bfloat16
AF = mybir.ActivationFunctionType
BF16NP = ml_dtypes.bfloat16

P = 128
E = 1024            # embedding dim
T = 2048            # sequence length
HC = 8              # heads per core
S = 64              # head dim
ET = E // P         # 8 contraction tiles over e
KT = T // P         # 16 key tiles
NPAIR = HC // 2     # 4 head pairs per core
QW = 512            # query-quarter width
NQ = T // QW        # 4 query quarters per pair
FC = NPAIR          # feature k-tiles for the output projection
SCALE = float(E ** -0.25)

N_CORES = 8


def build_nc():
    nc = bacc.Bacc("TRN2", target_bir_lowering=False, debug=False,
                   enable_asserts=False)

    xt = nc.dram_tensor("xt", [E, T], BF16, kind="ExternalInput").ap()
    wk = nc.dram_tensor("wk", [E, P * NPAIR], BF16, kind="ExternalInput").ap()
    wq = nc.dram_tensor("wq", [E, P * NPAIR], BF16, kind="ExternalInput").ap()
    wv = nc.dram_tensor("wv", [E, P * NPAIR], BF16, kind="ExternalInput").ap()
    wu = nc.dram_tensor("wu", [P * NPAIR, E], BF16, kind="ExternalInput").ap()
    bu = nc.dram_tensor("bu", [1, E], FP32, kind="ExternalInput").ap()
    out = nc.dram_tensor("out", [T, E], FP32, kind="ExternalOutput").ap()

    with tile.TileContext(nc) as tc:
        _build_kernel(tc, nc, xt, wk, wq, wv, wu, bu, out)
    nc.compile()
    return nc


def _build_kernel(tc, nc, xt, wk, wq, wv, wu, bu, out):
    with (
        tc.tile_pool(name="main", bufs=1) as mp,
        tc.tile_pool(name="psum", bufs=1, space="PSUM") as pspool,
        tc.tile_pool(name="dram", bufs=1, space="DRAM") as drampool,
    ):
        vv = mp.tile([P, KT, HC, S + 1], BF16, tag="vv", name="vv")
        yt = mp.tile([P, FC, T], BF16, tag="yt", name="yt")
        bub = mp.tile([P, E], FP32, tag="bub", name="bub")
        xt_sb = mp.tile([P, ET, T], BF16, tag="xtb", name="xtb")

        def avt(name):
            return pspool.tile([P, QW], FP32, tag="av", bufs=4, name=name)

        def psum_combine(dst_ap, h0_ap, h1_ap, nm, rows=P):
            # DVE reads at most one PSUM operand per instruction: stage one
            # half in SBUF, then add
            cmb = mp.tile([P, QW], FP32, tag="cmb", bufs=3, name=f"cmb{nm}")
            nc.vector.tensor_copy(out=cmb[0:rows, :], in_=h1_ap)
            nc.vector.tensor_add(out=dst_ap, in0=h0_ap, in1=cmb[0:rows, :])

        def load_w(dram_ap, j, tag, name):
            wt = mp.tile([P, ET, P], BF16, tag=tag, bufs=2, name=name)
            nc.sync.dma_start(
                wt[:], dram_ap[:, j * P:(j + 1) * P].rearrange(
                    "(ko p) m -> p ko m", p=P))
            return wt

        def proj_chunk(wb, dst, t0, uname):
            """dst[:, t0:t0+512] = wb.T @ X^T chunk, via two 64-row
            half-chains on alternating row groups (packed into one
            score-ring tile) + a DVE combine."""
            ph = pspool.tile([P, 2 * QW], FP32, tag="ps", bufs=2,
                             name=f"pp{uname}_{t0}")
            for k in range(ET):
                for h in range(2):
                    nc.tensor.matmul(
                        ph[:, h * QW:(h + 1) * QW],
                        lhsT=wb[h * S:(h + 1) * S, k, :],
                        rhs=xt_sb[h * S:(h + 1) * S, k, t0:t0 + QW],
                        start=(k == 0), stop=(k == ET - 1))
            psum_combine(dst[:, t0:t0 + QW], ph[:, 0:QW], ph[:, QW:2 * QW],
                         f"p{uname}{t0}")

        def proj_chunk_av(wb, dst, t0, uname):
            """Same, on the av-ring (startup only, before the AV
            accumulators exist)."""
            ph = [avt(f"pa{uname}_{t0}_{h}") for h in range(2)]
            for k in range(ET):
                for h in range(2):
                    nc.tensor.matmul(
                        ph[h][:],
                        lhsT=wb[h * S:(h + 1) * S, k, :],
                        rhs=xt_sb[h * S:(h + 1) * S, k, t0:t0 + QW],
                        start=(k == 0), stop=(k == ET - 1))
            psum_combine(dst[:, t0:t0 + QW], ph[0][:], ph[1][:],
                         f"p{uname}{t0}")

        def v_finish(mt, in0, in1, both_psum=True):
            vs = mp.tile([P, QW], FP32, tag="vs", bufs=2, name=f"vs{mt}")
            if both_psum:
                psum_combine(vs[:], in0, in1, f"v{mt}")
            else:
                nc.vector.tensor_add(out=vs[:], in0=in0, in1=in1)
            nc.vector.tensor_copy(
                out=vv[:, mt, :, 0:S],
                in_=vs[:].rearrange("p (h s) -> p h s", s=S))
            nc.vector.memset(vv[:, mt, :, S:S + 1], 1.0)

        def v_chunk(mt):
            """vv tile mt (one 128-token block, 512 features + ones col),
            halves on the av-ring (quarter 0 only — ring is free there)."""
            ph = [avt(f"pv{mt}_{h}") for h in range(2)]
            for k in range(ET):
                for h in range(2):
                    nc.tensor.matmul(
                        ph[h][:],
                        lhsT=xt_sb[h * S:(h + 1) * S, k,
                                   mt * P:(mt + 1) * P],
                        rhs=wvb[h * S:(h + 1) * S, k, :],
                        start=(k == 0), stop=(k == ET - 1))
            v_finish(mt, ph[0][:], ph[1][:])

        def v_chunk_ps(mt):
            """Same, but halves packed into one score-ring tile (quarter 1:
            the av-ring is held by the lagged AV accumulators there)."""
            ph = pspool.tile([P, 2 * QW], FP32, tag="ps", bufs=2,
                             name=f"pvp{mt}")
            for k in range(ET):
                for h in range(2):
                    nc.tensor.matmul(
                        ph[:, h * QW:(h + 1) * QW],
                        lhsT=xt_sb[h * S:(h + 1) * S, k,
                                   mt * P:(mt + 1) * P],
                        rhs=wvb[h * S:(h + 1) * S, k, :],
                        start=(k == 0), stop=(k == ET - 1))
            v_finish(mt, ph[:, 0:QW], ph[:, QW:2 * QW])

        def out_unit(m, n0, ot):
            """ot[:, n0:n0+512] = out-projection chunk for token tile m;
            contraction halves packed into one score-ring tile."""
            ph = pspool.tile([P, 2 * QW], FP32, tag="ps", bufs=2,
                             name=f"po{m}_{n0}")
            for k in range(FC):
                for h in range(2):
                    nc.tensor.matmul(
                        ph[:, h * QW:(h + 1) * QW],
                        lhsT=yt[h * S:(h + 1) * S, k, m * P:(m + 1) * P],
                        rhs=wub[h * S:(h + 1) * S, k, n0:n0 + QW],
                        start=(k == 0), stop=(k == FC - 1))
            tsum = mp.tile([P, QW], FP32, tag="osum", bufs=2,
                           name=f"os{m}_{n0}")
            psum_combine(tsum[:], ph[:, 0:QW], ph[:, QW:2 * QW],
                         f"o{m}_{n0}")
            nc.vector.tensor_add(out=ot[:, n0:n0 + QW], in0=tsum[:],
                                 in1=bub[:, n0:n0 + QW])

        def emit_out_tile(m):
            ot = mp.tile([P, E], FP32, tag="ot", bufs=2, name=f"ot{m}")
            for n0 in (0, QW):
                out_unit(m, n0, ot)
            nc.sync.dma_start(out[m * P:(m + 1) * P, :], ot[:])

        def emit_normalize(j, qq, avh):
            # combine AV halves (also evicts the banks), batch the two
            # denominator rows into a [128, 8] partition-packed reciprocal
            # via a DRAM bounce, broadcast back, normalize into yt
            q0 = qq * QW
            yraws = []
            for par in range(2):
                yraw = mp.tile([S + 1, QW], FP32, tag=f"yraw{par}",
                               bufs=2, name=f"yraw{j}_{qq}_{par}")
                psum_combine(yraw[:], avh[par][0][0:S + 1, :],
                             avh[par][1][0:S + 1, :], f"n{j}_{qq}_{par}",
                             rows=S + 1)
                yraws.append(yraw)
            db = drampool.tile([1, 2 * QW], FP32, tag="db", bufs=2,
                               name=f"db{j}_{qq}")
            for par in range(2):
                nc.sync.dma_start(db[:, par * QW:(par + 1) * QW],
                                  yraws[par][S:S + 1, :])
            rin = mp.tile([P, 8], FP32, tag="rin", bufs=2,
                          name=f"rin{j}_{qq}")
            nc.sync.dma_start(
                rin[:], db[0:1, :].rearrange("a (p f) -> (a p) f", p=P))
            rcp = mp.tile([P, 8], FP32, tag="rcp", bufs=2,
                          name=f"rcp{j}_{qq}")
            nc.vector.reciprocal_approx_fast(out=rcp[:], in_=rin[:])
            db2 = drampool.tile([1, 2 * QW], FP32, tag="db2", bufs=2,
                                name=f"db2{j}_{qq}")
            nc.sync.dma_start(
                db2[0:1, :].rearrange("a (p f) -> (a p) f", p=P), rcp[:])
            for par in range(2):
                rbc = mp.tile([S, QW], FP32, tag="rbc", bufs=2,
                              name=f"rbc{j}_{qq}_{par}")
                nc.sync.dma_start(
                    rbc[:],
                    db2[0:1, par * QW:(par + 1) * QW].to_broadcast([S, QW]))
                if par == 0:
                    nc.vector.tensor_mul(out=yt[0:S, j, q0:q0 + QW],
                                         in0=yraws[par][0:S, :], in1=rbc[:])
                else:
                    tmp = mp.tile([S, QW], BF16, tag="tmp", bufs=2,
                                  name=f"tmp{j}_{qq}")
                    nc.vector.tensor_mul(out=tmp[:],
                                         in0=yraws[par][0:S, :], in1=rbc[:])
                    nc.sync.dma_start(yt[S:P, j, q0:q0 + QW], tmp[:])

        # --- input DMAs
        wkj = {0: load_w(wk, 0, "wkj", "wk0")}
        wqj = {0: load_w(wq, 0, "wqj", "wq0")}
        for k in range(ET):
            nc.sync.dma_start(xt_sb[:, k, :], xt[k * P:(k + 1) * P, :])
        wvb = mp.tile([P, ET, P * NPAIR], BF16, tag="wvb", name="wvb")
        nc.sync.dma_start(wvb[:], wv.rearrange("(ko p) m -> p ko m", p=P))
        nc.sync.dma_start(bub[:], bu.to_broadcast([P, E]))
        wub = mp.tile([P, FC, E], BF16, tag="wub", name="wub")
        nc.sync.dma_start(wub[:], wu.rearrange("(ko p) m -> p ko m", p=P))

        kq = {0: (mp.tile([P, T], BF16, tag="ktj", bufs=2, name="kt0"),
                  mp.tile([P, T], BF16, tag="qtj", bufs=2, name="qt0"))}

        # pair-0 K/Q startup chains on the av-ring (first score needs only
        # kt c0 + qt c0, so emit those first)
        proj_chunk_av(wkj[0], kq[0][0], 0, "k0")
        proj_chunk_av(wqj[0], kq[0][1], 0, "q0")
        for t0 in range(QW, T, QW):
            proj_chunk_av(wkj[0], kq[0][0], t0, "k0")

        # background work queues -------------------------------------------
        # mid_fill[t] = thunks emitted inside quarter t at slots 6 and 11
        QTOT = NPAIR * NQ  # 16 quarters
        mid_fill = [[] for _ in range(QTOT)]

        def defer(t, fn):
            mid_fill[t].append(fn)

        def mk_w(jn):
            wkj[jn] = load_w(wk, jn, "wkj", f"wk{jn}")
            wqj[jn] = load_w(wq, jn, "wqj", f"wq{jn}")
            kq[jn] = (mp.tile([P, T], BF16, tag="ktj", bufs=2,
                              name=f"kt{jn}"),
                      mp.tile([P, T], BF16, tag="qtj", bufs=2,
                              name=f"qt{jn}"))

        def kc(jn, c):
            return lambda: proj_chunk(wkj[jn], kq[jn][0], c * QW, f"k{jn}")

        def qc(jn, c):
            return lambda: proj_chunk(wqj[jn], kq[jn][1], c * QW, f"q{jn}")

        # one chunk per designated slot, load-balanced so every chunk lands
        # before its deadline (kt all + qt c0 before quarter 4*jn; qt chunk
        # c before quarter 4*jn + c). Quarter 0 has no lagged AV work, so it
        # absorbs extra chunks; quarter 1 carries the V spill and gets none.
        defer(0, lambda: mk_w(1))
        defer(0, qc(0, 1))
        defer(0, qc(0, 2))
        defer(0, qc(0, 3))
        defer(0, kc(1, 0))
        defer(2, kc(1, 1))
        defer(2, kc(1, 2))
        defer(3, lambda: mk_w(2))
        defer(3, kc(1, 3))
        defer(3, qc(1, 0))
        defer(4, qc(1, 1))
        defer(4, kc(2, 0))
        defer(5, qc(1, 2))
        defer(5, kc(2, 1))
        defer(5, kc(2, 2))
        defer(6, qc(1, 3))
        defer(6, kc(2, 3))
        defer(7, lambda: mk_w(3))
        defer(7, qc(2, 0))
        defer(8, qc(2, 1))
        defer(8, kc(3, 0))
        defer(9, qc(2, 2))
        defer(9, kc(3, 1))
        defer(9, kc(3, 2))
        defer(10, qc(2, 3))
        defer(10, kc(3, 3))
        defer(11, qc(3, 0))
        defer(12, qc(3, 1))
        defer(13, qc(3, 2))
        defer(14, qc(3, 3))

        # --- quarter loop: uniform lag-one-quarter schedule ---------------
        prev = None  # (j, qq, pts list) whose AVs run in this quarter
        for t in range(QTOT):
            j, qq = t // NQ, t % NQ
            q0 = qq * QW
            ktj, qtj = kq[j]
            pts = []
            avh = None
            if prev is not None:
                avh = [[avt(f"av{t}_{par}_{h}") for h in range(2)]
                       for par in range(2)]
            # out-projection units for the last pair's quarters, two
            # normalizes behind (their yt slice is complete by then)
            outq = []
            if j == NPAIR - 1 and qq >= 2:
                for m in range((qq - 2) * NQ, (qq - 1) * NQ):
                    ot = mp.tile([P, E], FP32, tag="ot", bufs=2,
                                 name=f"ot{m}")
                    outq.append((m, 0, ot, False))
                    outq.append((m, QW, ot, True))
            # one background chunk per designated slot (odd slots; V uses
            # the even ones in quarters 0-1)
            fills = list(mid_fill[t])
            fill_at = {}
            fslots = (5, 7, 9, 11, 13) if t == 0 else (3, 5, 8, 11, 13)
            for n, fn in enumerate(fills):
                fill_at.setdefault(fslots[n % len(fslots)], []).append(fn)
            for i in range(KT):
                ps = pspool.tile([P, 2 * QW], FP32, tag="ps", bufs=2,
                                 name=f"s{t}_{i}")
                for par in range(2):
                    lo = par * S
                    nc.tensor.matmul(
                        ps[:, par * QW:(par + 1) * QW],
                        lhsT=ktj[lo:lo + S, i * P:(i + 1) * P],
                        rhs=qtj[lo:lo + S, q0:q0 + QW],
                        start=True, stop=True)
                pt = mp.tile([P, 2 * QW], BF16, tag="pt", bufs=20,
                             name=f"p{t}_{i}")
                nc.scalar.activation(pt[:], ps[:], AF.Exp)
                pts.append(pt)
                # V projection fills the slack of the first two quarters
                # (emitted before the AVs that will consume those vv tiles)
                if t == 0 and i % 2 == 0:
                    v_chunk(i // 2)
                elif t == 1 and i % 2 == 0:
                    v_chunk_ps(8 + i // 2)
                if prev is not None:
                    # lagged AV for the previous quarter, same key tile i
                    pj, pqq, ppts = prev
                    for par in range(2):
                        h2 = 2 * pj + par
                        for hf in range(2):
                            nc.tensor.matmul(
                                avh[par][hf][0:S + 1, :],
                                lhsT=vv[hf * S:(hf + 1) * S, i, h2, :],
                                rhs=ppts[i][hf * S:(hf + 1) * S,
                                            par * QW:(par + 1) * QW],
                                start=(i == 0), stop=(i == KT - 1))
                # background projection chunks, one per designated slot
                for fn in fill_at.get(i, ()):
                    fn()
                # one output-projection unit per odd slot
                if outq and i % 2 == 1:
                    m, n0, ot, last = outq.pop(0)
                    out_unit(m, n0, ot)
                    if last:
                        nc.sync.dma_start(out[m * P:(m + 1) * P, :], ot[:])
            # leftover output units (none expected in steady quarters)
            for m, n0, ot, last in outq:
                out_unit(m, n0, ot)
                if last:
                    nc.sync.dma_start(out[m * P:(m + 1) * P, :], ot[:])
            # boundary: previous quarter's normalize (DVE/DMA only)
            if prev is not None:
                emit_normalize(prev[0], prev[1], avh)
            prev = (j, qq, pts)

        # --- tail: last quarter's AVs, normalize, final output tiles
        j, qq, ppts = prev
        avh = [[avt(f"avT_{par}_{h}") for h in range(2)]
               for par in range(2)]
        for i in range(KT):
            for par in range(2):
                h2 = 2 * j + par
                for hf in range(2):
                    nc.tensor.matmul(
                        avh[par][hf][0:S + 1, :],
                        lhsT=vv[hf * S:(hf + 1) * S, i, h2, :],
                        rhs=ppts[i][hf * S:(hf + 1) * S,
                                    par * QW:(par + 1) * QW],
                        start=(i == 0), stop=(i == KT - 1))
        emit_normalize(j, qq, avh)
        for m in range((NQ - 2) * NQ, NQ * NQ):
            emit_out_tile(m)


_NC_CACHE = {}


def _get_nc():
    if "nc" not in _NC_CACHE:
        _NC_CACHE["nc"] = build_nc()
    return _NC_CACHE["nc"]


def make_in_maps(X, W_k, W_q, W_v, W_u, b_u):
    X = np.asarray(X, np.float32)
    b = X.shape[0]
    HW = P * NPAIR  # 512 features per head-half
    wk_t = (np.asarray(W_k, np.float32).T * SCALE).astype(BF16NP)
    wq_t = (np.asarray(W_q, np.float32).T * SCALE).astype(BF16NP)
    wv_t = (np.asarray(W_v, np.float32).T * SCALE).astype(BF16NP)
    wu_t = np.asarray(W_u, np.float32).T.astype(BF16NP)
    bu2 = np.ascontiguousarray(np.asarray(b_u, np.float32).reshape(1, E))
    bu_zero = np.zeros((1, E), np.float32)
    wk_s = [np.ascontiguousarray(wk_t[:, hh * HW:(hh + 1) * HW])
            for hh in range(2)]
    wq_s = [np.ascontiguousarray(wq_t[:, hh * HW:(hh + 1) * HW])
            for hh in range(2)]
    wv_s = [np.ascontiguousarray(wv_t[:, hh * HW:(hh + 1) * HW])
            for hh in range(2)]
    wu_s = [np.ascontiguousarray(wu_t[hh * HW:(hh + 1) * HW, :])
            for hh in range(2)]
    xts = [np.ascontiguousarray(X[bi].T).astype(BF16NP) for bi in range(b)]
    in_maps = []
    for c in range(N_CORES):
        bi, hh = c // 2, c % 2
        in_maps.append({
            "xt": xts[bi],
            "wk": wk_s[hh], "wq": wq_s[hh], "wv": wv_s[hh],
            "wu": wu_s[hh],
            "bu": bu2 if hh == 0 else bu_zero,
        })
    return in_maps


def run(inputs, trace=False, **kwargs):
    """Run on hardware; returns (full output, BassKernelResults)."""
    X = np.asarray(inputs["X"], np.float32)
    b, t, e = X.shape
    nc = _get_nc()
    in_maps = make_in_maps(X, inputs["W_k"], inputs["W_q"], inputs["W_v"],
                           inputs["W_u"], inputs["b_u"])
    res = run_bass_kernel_spmd(nc, in_maps, core_ids=list(range(N_CORES)),
                               trace=trace, **kwargs)
    full = np.empty((b, t, e), np.float32)
    for bi in range(b):
        full[bi] = res.results[2 * bi]["out"] + res.results[2 * bi + 1]["out"]
    return full, res


def kernel(**inputs):
    full, _ = run(inputs)
    return full


# revision 25
# speedup vs baseline: 1.3377x; 1.2962x over previous
# Multi-head attention kernel for Trainium2 (8 NeuronCores, SPMD).
#
# Problem (hardcoded): X[4, 2048, 1024], W_k/W_q/W_v/W_u[1024, 1024], b_u[1024]
#   K = (X @ W_k.T) * s ; Q = (X @ W_q.T) * s ; V = (X @ W_v.T) * s   (s = 1024**-0.25)
#   S = Q @ K.T per head (16 heads, head_dim 64); P = softmax(S); Y = P @ V
#   out = Y @ W_u.T + b_u
#
# Sharding: core c = (batch c//2, head-half c%2). Each core computes K/Q/V for
# its 8 heads over the full sequence of its batch and the matching 512-wide
# slice of the output projection, producing a PARTIAL output [2048, 1024]
# (feature-contraction split). Host unshard = add the two partials per batch
# (column-sharded projection reduce); b_u is added on the hh==0 core only.
#
# Design (from trace analysis): exp on the Scalar engine is the hard floor
# (~295us/core: 33.5M score elements at 1 elem/cycle/lane @1.2GHz, invariant
# under sharding), so the kernel keeps ACT saturated and fits the PE work
# underneath it. Weights arrive pre-scaled/pre-transposed/bf16 from the host.
# The attention q axis runs in 512-wide quarters with both heads of a pair
# packed into one [128, 1024] score PSUM tile so each EXP activation covers
# both heads (N=1024 amortizes the ~350-cycle ACTIVATE overhead). Softmax
# denominators ride as row 64 of the AV matmul (ones column in V); their
# reciprocal is computed partition-packed ([128, 8] via a DRAM bounce)
# instead of on a 1-partition row, which the DVE executes pathologically
# slowly. The output projection splits each contraction into two 64-row
# halves on alternating row groups (the hardware pulls row-disjoint weight
# loads ahead of in-flight matmuls, so the halves stream concurrently).
#
# Per-core layout (PE contracts on partitions):
#   X^T  [e, t]      bf16 from host
#   K^T/Q^T [128, t] per head pair: head A on partitions 0..63, head B 64..127
#   V    [t, h, 65]  token-major, 65th column = ones (softmax denominator)
#   Y^T  [128, 4, t] feature-major (pair -> partition block), normalized
#   out  [t, e]      partial projection, fp32

import numpy as np
import ml_dtypes

import concourse.bacc as bacc
import concourse.mybir as mybir
import concourse.tile as tile
from concourse.bass_utils import run_bass_kernel_spmd

FP32 = mybir.dt.float32
BF16 = mybir.dt.bfloat16
AF = mybir.ActivationFunctionType
BF16NP = ml_dtypes.bfloat16

P = 128
E = 1024            # embedding dim
T = 2048            # sequence length
HC = 8              # heads per core
S = 64              # head dim
ET = E // P         # 8 contraction tiles over e
KT = T // P         # 16 key tiles
NPAIR = HC // 2     # 4 head pairs per core
QW = 512            # query-quarter width
NQ = T // QW        # 4 query quarters
FC = NPAIR          # feature k-tiles for the output projection (4)
SCALE = float(E ** -0.25)

N_CORES = 8


def _chunks(total, step):
    return [(o, min(step, total - o)) for o in range(0, total, step)]


def build_nc():
    nc = bacc.Bacc("TRN2", target_bir_lowering=False, debug=False,
                   enable_asserts=False)

    xt = nc.dram_tensor("xt", [E, T], BF16, kind="ExternalInput").ap()
    wk = nc.dram_tensor("wk", [E, P * NPAIR], BF16, kind="ExternalInput").ap()
    wq = nc.dram_tensor("wq", [E, P * NPAIR], BF16, kind="ExternalInput").ap()
    wv = nc.dram_tensor("wv", [E, P * NPAIR], BF16, kind="ExternalInput").ap()
    wu = nc.dram_tensor("wu", [P * NPAIR, E], BF16, kind="ExternalInput").ap()
    bu = nc.dram_tensor("bu", [1, E], FP32, kind="ExternalInput").ap()
    out = nc.dram_tensor("out", [T, E], FP32, kind="ExternalOutput").ap()

    with tile.TileContext(nc) as tc:
        _build_kernel(tc, nc, xt, wk, wq, wv, wu, bu, out)
    nc.compile()
    return nc


def _build_kernel(tc, nc, xt, wk, wq, wv, wu, bu, out):
    with (
        tc.tile_pool(name="main", bufs=1) as mp,
        tc.tile_pool(name="psum", bufs=1, space="PSUM") as pspool,
        tc.tile_pool(name="dram", bufs=1, space="DRAM") as drampool,
    ):
        vv = mp.tile([P, KT, HC, S + 1], BF16, tag="vv", name="vv")
        yt = mp.tile([P, FC, T], BF16, tag="yt", name="yt")
        bub = mp.tile([P, E], FP32, tag="bub", name="bub")

        # --- per-pair K/Q weight load + projection (kt/qt [128, T]:
        # head 2j on partitions 0..63, head 2j+1 on 64..127)
        wts = {}

        def load_wkq(j):
            wkj = mp.tile([P, ET, P], BF16, tag="wkj", bufs=2, name=f"wk{j}")
            wqj = mp.tile([P, ET, P], BF16, tag="wqj", bufs=2, name=f"wq{j}")
            nc.sync.dma_start(
                wkj[:], wk[:, j * P:(j + 1) * P].rearrange(
                    "(ko p) m -> p ko m", p=P))
            nc.sync.dma_start(
                wqj[:], wq[:, j * P:(j + 1) * P].rearrange(
                    "(ko p) m -> p ko m", p=P))
            wts[j] = (wkj, wqj)

        def emit_proj_one(j, which, dst):
            wb = wts[j][which]
            uname = "kq"[which]
            for t0, tw in _chunks(T, 1024):
                ps = pspool.tile([P, 1024], FP32, tag="ps", bufs=3,
                                 name=f"ps{uname}{j}_{t0}")
                for n0, nw in _chunks(tw, 512):
                    for k in range(ET):
                        nc.tensor.matmul(
                            ps[:, n0:n0 + nw],
                            lhsT=wb[:, k, :],
                            rhs=xt_sb[:, k, t0 + n0:t0 + n0 + nw],
                            start=(k == 0), stop=(k == ET - 1))
                nc.vector.tensor_copy(out=dst[:, t0:t0 + tw], in_=ps[:])

        # --- X^T (bf16 direct from host), chunked per k-tile for early start
        xt_sb = mp.tile([P, ET, T], BF16, tag="xtb", name="xtb")
        load_wkq(0)
        for k in range(ET):
            nc.sync.dma_start(xt_sb[:, k, :], xt[k * P:(k + 1) * P, :])
        wvb = mp.tile([P, ET, P * NPAIR], BF16, tag="wvb", name="wvb")
        nc.sync.dma_start(wvb[:], wv.rearrange("(ko p) m -> p ko m", p=P))
        nc.sync.dma_start(bub[:], bu.to_broadcast([P, E]))

        # pair-0 K/Q projection emitted first (feeds the first scores)
        kq = {0: (mp.tile([P, T], BF16, tag="ktj", bufs=2, name="kt0"),
                  mp.tile([P, T], BF16, tag="qtj", bufs=2, name="qt0"))}
        emit_proj_one(0, 0, kq[0][0])
        emit_proj_one(0, 1, kq[0][1])

        # --- V projection -> vv[t, h, 0:64] token-major + ones column,
        # two token-tiles per PSUM tile
        for mt in range(0, KT, 2):
            ps = pspool.tile([P, 1024], FP32, tag="ps", bufs=3,
                             name=f"psv{mt}")
            for sub in range(2):
                for k in range(ET):
                    nc.tensor.matmul(
                        ps[:, sub * 512:(sub + 1) * 512],
                        lhsT=xt_sb[:, k, (mt + sub) * P:(mt + sub + 1) * P],
                        rhs=wvb[:, k, :],
                        start=(k == 0), stop=(k == ET - 1))
            nc.vector.tensor_copy(
                out=vv[:, mt:mt + 2, :, 0:S],
                in_=ps[:].rearrange("p (m h s) -> p m h s", m=2, s=S))
            nc.vector.memset(vv[:, mt:mt + 2, :, S:S + 1], 1.0)

        # output projection weights, DMA'd early so the tail never waits
        wub = mp.tile([P, FC, E], BF16, tag="wub", name="wub")
        nc.sync.dma_start(wub[:], wu.rearrange("(ko p) m -> p ko m", p=P))

        def emit_out_tile(m):
            # output projection token tile; contraction split into 64-row
            # halves on alternating row groups (halves stream concurrently,
            # ~2x a serial accumulation chain), packed into one ps tile
            ot = mp.tile([P, E], FP32, tag="ot", bufs=2, name=f"ot{m}")
            for n0 in (0, 512):
                ph = pspool.tile([P, 1024], FP32, tag="ps", bufs=3,
                                 name=f"po{m}_{n0}")
                for k in range(FC):
                    for h in range(2):
                        nc.tensor.matmul(
                            ph[:, h * 512:(h + 1) * 512],
                            lhsT=yt[h * S:(h + 1) * S, k,
                                    m * P:(m + 1) * P],
                            rhs=wub[h * S:(h + 1) * S, k, n0:n0 + 512],
                            start=(k == 0), stop=(k == FC - 1))
                tsum = mp.tile([P, 512], FP32, tag="osum", bufs=2,
                               name=f"os{m}_{n0}")
                nc.vector.tensor_copy(out=tsum[:], in_=ph[:, 512:1024])
                nc.vector.tensor_add(out=tsum[:], in0=ph[:, 0:512],
                                     in1=tsum[:])
                nc.vector.tensor_add(out=ot[:, n0:n0 + 512], in0=tsum[:],
                                     in1=bub[:, n0:n0 + 512])
            nc.sync.dma_start(out[m * P:(m + 1) * P, :], ot[:])

        # --- head pairs; per pair, 4 query-quarters of 512
        for j in range(NPAIR):
            ktj, qtj = kq.pop(j)
            for qq in range(NQ):
                q0 = qq * QW
                # AV accumulators: one PSUM bank per parity, held over the
                # full key loop (ones column gives the denominator as row 64)
                avs = [pspool.tile([P, QW], FP32, tag="av", bufs=2,
                                   name=f"av{j}_{qq}_{par}")
                       for par in range(2)]
                for i in range(KT):
                    ps = pspool.tile([P, 1024], FP32, tag="ps", bufs=3,
                                     name=f"s{j}_{qq}_{i}")
                    for par in range(2):
                        lo = par * S
                        nc.tensor.matmul(
                            ps[:, par * QW:(par + 1) * QW],
                            lhsT=ktj[lo:lo + S, i * P:(i + 1) * P],
                            rhs=qtj[lo:lo + S, q0:q0 + QW],
                            start=True, stop=True)
                    pt = mp.tile([P, 1024], BF16, tag="pt", bufs=16,
                                 name=f"p{j}_{qq}_{i}")
                    nc.scalar.activation(pt[:], ps[:], AF.Exp)
                    for par in range(2):
                        nc.tensor.matmul(
                            avs[par][0:S + 1, :],
                            lhsT=vv[:, i, 2 * j + par, :],
                            rhs=pt[:, par * QW:(par + 1) * QW],
                            start=(i == 0), stop=(i == KT - 1))
                # prefetch next pair's K/Q projection into the PE's slack,
                # split across two quarter boundaries (kt after qq0, qt
                # after qq1) so each block is half the size
                if j + 1 < NPAIR:
                    if qq == 0:
                        load_wkq(j + 1)
                        kq[j + 1] = (
                            mp.tile([P, T], BF16, tag="ktj", bufs=2,
                                    name=f"kt{j+1}"),
                            mp.tile([P, T], BF16, tag="qtj", bufs=2,
                                    name=f"qt{j+1}"))
                        emit_proj_one(j + 1, 0, kq[j + 1][0])
                    elif qq == 1:
                        emit_proj_one(j + 1, 1, kq[j + 1][1])

                # normalize: evict AV banks, batch both parities' denominator
                # rows into a [128, 8] partition-packed reciprocal via a DRAM
                # bounce, broadcast back, multiply into yt
                yraws = []
                for par in range(2):
                    yraw = mp.tile([S + 1, QW], FP32, tag=f"yraw{par}",
                                   bufs=2, name=f"yraw{j}_{qq}_{par}")
                    nc.vector.tensor_copy(out=yraw[:], in_=avs[par][0:S + 1, :])
                    yraws.append(yraw)
                db = drampool.tile([1, 1024], FP32, tag="db", bufs=4,
                                   name=f"db{j}_{qq}")
                for par in range(2):
                    nc.sync.dma_start(db[:, par * QW:(par + 1) * QW],
                                      yraws[par][S:S + 1, :])
                rin = mp.tile([P, 8], FP32, tag="rin", bufs=2,
                              name=f"rin{j}_{qq}")
                nc.sync.dma_start(
                    rin[:], db[0:1, :].rearrange("a (p f) -> (a p) f", p=P))
                rcp = mp.tile([P, 8], FP32, tag="rcp", bufs=2,
                              name=f"rcp{j}_{qq}")
                nc.vector.reciprocal_approx_fast(out=rcp[:], in_=rin[:])
                db2 = drampool.tile([1, 1024], FP32, tag="db2", bufs=4,
                                    name=f"db2{j}_{qq}")
                nc.sync.dma_start(
                    db2[0:1, :].rearrange("a (p f) -> (a p) f", p=P), rcp[:])
                for par in range(2):
                    rbc = mp.tile([S, QW], FP32, tag="rbc", bufs=2,
                                  name=f"rbc{j}_{qq}_{par}")
                    nc.sync.dma_start(
                        rbc[:],
                        db2[0:1, par * QW:(par + 1) * QW].to_broadcast(
                            [S, QW]))
                    if par == 0:
                        nc.vector.tensor_mul(out=yt[0:S, j, q0:q0 + QW],
                                             in0=yraws[par][0:S, :],
                                             in1=rbc[:])
                    else:
                        tmp = mp.tile([S, QW], BF16, tag="tmp", bufs=2,
                                      name=f"tmp{j}_{qq}")
                        nc.vector.tensor_mul(out=tmp[:],
                                             in0=yraws[par][0:S, :],
                                             in1=rbc[:])
                        nc.sync.dma_start(yt[S:P, j, q0:q0 + QW], tmp[:])

        # --- output projection out[q, e'] = Y^T.T @ W_u^T + b_u (partial)
        for m in range(T // P):
            emit_out_tile(m)


_NC_CACHE = {}


def _get_nc():
    if "nc" not in _NC_CACHE:
        _NC_CACHE["nc"] = build_nc()
    return _NC_CACHE["nc"]


def make_in_maps(X, W_k, W_q, W_v, W_u, b_u):
    X = np.asarray(X, np.float32)
    b = X.shape[0]
    HW = P * NPAIR  # 512 features per head-half
    # pre-transpose, pre-scale, cast to bf16 on host (same numerics as the
    # on-device scale+cast it replaces)
    wk_t = (np.asarray(W_k, np.float32).T * SCALE).astype(BF16NP)
    wq_t = (np.asarray(W_q, np.float32).T * SCALE).astype(BF16NP)
    wv_t = (np.asarray(W_v, np.float32).T * SCALE).astype(BF16NP)
    wu_t = np.asarray(W_u, np.float32).T.astype(BF16NP)
    bu2 = np.ascontiguousarray(np.asarray(b_u, np.float32).reshape(1, E))
    bu_zero = np.zeros((1, E), np.float32)
    wk_s = [np.ascontiguousarray(wk_t[:, hh * HW:(hh + 1) * HW])
            for hh in range(2)]
    wq_s = [np.ascontiguousarray(wq_t[:, hh * HW:(hh + 1) * HW])
            for hh in range(2)]
    wv_s = [np.ascontiguousarray(wv_t[:, hh * HW:(hh + 1) * HW])
            for hh in range(2)]
    wu_s = [np.ascontiguousarray(wu_t[hh * HW:(hh + 1) * HW, :])
            for hh in range(2)]
    xts = [np.ascontiguousarray(X[bi].T).astype(BF16NP) for bi in range(b)]
    in_maps = []
    for c in range(N_CORES):
        bi, hh = c // 2, c % 2
        in_maps.append({
            "xt": xts[bi],
            "wk": wk_s[hh], "wq": wq_s[hh], "wv": wv_s[hh],
            "wu": wu_s[hh],
            "bu": bu2 if hh == 0 else bu_zero,
        })
    return in_maps


def run(inputs, trace=False, **kwargs):
    """Run on hardware; returns (full output, BassKernelResults)."""
    X = np.asarray(inputs["X"], np.float32)
    b, t, e = X.shape
    nc = _get_nc()
    in_maps = make_in_maps(X, inputs["W_k"], inputs["W_q"], inputs["W_v"],
                           inputs["W_u"], inputs["b_u"])
    res = run_bass_kernel_spmd(nc, in_maps, core_ids=list(range(N_CORES)),
                               trace=trace, **kwargs)
    full = np.empty((b, t, e), np.float32)
    for bi in range(b):
        full[bi] = res.results[2 * bi]["out"] + res.results[2 * bi + 1]["out"]
    return full, res


def kernel(**inputs):
    full, _ = run(inputs)
    return full
